# revision 9
# baseline (speedup 1.0000x reference)
"""Fused conv3x3 -> GroupNorm(16) -> channel scale -> maxpool2x2 -> clamp[0,1]
Trainium2 Bass kernel, data-parallel over batch on 8 NeuronCores.

Input  x [32, 64, 128, 128] f32  -> output [32, 128, 63, 63] f32.
Each core handles 4 samples.

Conv strategy: tap-wise matmuls with contraction over (cin, kh-pair) on the
128 SBUF partitions. SBUF x-buffer layout: partition p = ci + 64*r holds
x[ci, row+r, :] (r in {0,1}), so one matmul with a [128, 128] stacked weight
covers taps (kh=0, kw) and (kh=1, kw) at once; kh=2 taps run as 64-partition
matmuls. 6 matmuls per 4 output rows, fp32r (full PE rate, ~1e-4 rel err).

GroupNorm: per-channel mean/var via one-pass bn_stats/bn_aggr on the DVE,
8-channel group reduction via a tiny block-diagonal-ones matmul (fp32),
conv bias folded analytically into the final per-channel affine
z = A*y + B, which is applied on the ScalarE with fused Relu (lower clamp).
Maxpool as two strided tensor_tensor max ops; upper clamp fused into a
tensor_scalar on the pooled tile.
"""

import numpy as np

import concourse.bacc as bacc
import concourse.mybir as mybir
import concourse.tile as tile
from concourse.bass_utils import run_bass_kernel_spmd

N_CORES = 8
B_FULL, CIN, H, W = 32, 64, 128, 128
COUT = 128
BPC = B_FULL // N_CORES  # samples per core
OH = OW = 126
PH = PW = 63
NG = 16  # groups
GSZ = COUT // NG  # 8 channels per group
EPS = 1e-5
S = OH * OW  # spatial size per sample

# (x_row0, n_xrows, out_row0, n_out_rows)
CHUNKS = [(0, 66, 0, 64), (64, 64, 64, 62)]
XROWS_MAX = 66

_CACHED = {}


def _build():
    if "nc" in _CACHED:
        return _CACHED["nc"]
    f32 = mybir.dt.float32
    f32r = mybir.dt.float32r
    AF = mybir.ActivationFunctionType
    OP = mybir.AluOpType

    nc = bacc.Bacc("TRN2", target_bir_lowering=False, debug=False)
    xs = nc.dram_tensor("xs", [BPC, CIN, H, W], f32r, kind="ExternalInput").ap()
    wp_d = nc.dram_tensor("wp", [3, 128, COUT], f32r, kind="ExternalInput").ap()
    ws_d = nc.dram_tensor("ws", [3, 64, COUT], f32r, kind="ExternalInput").ap()
    cb_d = nc.dram_tensor("cb", [COUT, 1], f32, kind="ExternalInput").ap()
    gs_d = nc.dram_tensor("gs", [COUT, 1], f32, kind="ExternalInput").ap()
    gbs_d = nc.dram_tensor("gbs", [COUT, 1], f32, kind="ExternalInput").ap()
    bones_d = nc.dram_tensor("bones", [COUT, COUT], f32, kind="ExternalInput").ap()
    out_d = nc.dram_tensor("out", [BPC, COUT, PH, PW], f32, kind="ExternalOutput").ap()

    with tile.TileContext(nc) as tc:
        with (
            tc.tile_pool(name="consts", bufs=1) as cpool,
            tc.tile_pool(name="xpool", bufs=2) as xpool,
            tc.tile_pool(name="ypool", bufs=1) as ypool,
            tc.tile_pool(name="stpool", bufs=2) as stpool,
            tc.tile_pool(name="phpool", bufs=1) as phpool,
            tc.tile_pool(name="pvpool", bufs=2) as pvpool,
            tc.tile_pool(name="cps", bufs=3, space="PSUM") as cps,
            tc.tile_pool(name="gps", bufs=1, space="PSUM") as gps,
        ):
            wp = cpool.tile([128, 3 * COUT], f32r, name="wp_t")
            ws = cpool.tile([64, 3 * COUT], f32r, name="ws_t")
            for kw in range(3):
                nc.sync.dma_start(wp[:, kw * COUT : (kw + 1) * COUT], wp_d[kw])
                nc.sync.dma_start(ws[:, kw * COUT : (kw + 1) * COUT], ws_d[kw])
            cb = cpool.tile([COUT, 1], f32, name="cb_t")
            nc.sync.dma_start(cb[:], cb_d[:])
            gs = cpool.tile([COUT, 1], f32, name="gs_t")
            nc.sync.dma_start(gs[:], gs_d[:])
            gbs = cpool.tile([COUT, 1], f32, name="gbs_t")
            nc.sync.dma_start(gbs[:], gbs_d[:])
            bones = cpool.tile([COUT, COUT], f32, name="bones_t")
            nc.sync.dma_start(bones[:], bones_d[:])
            zeros1 = cpool.tile([COUT, 1], f32, name="zeros1")
            nc.vector.memset(zeros1[:], 0.0)

            for b in range(BPC):
                y_raw = ypool.tile([128, S], f32, tag="y", name="y_raw")
                stats = stpool.tile([128, 32, 6], f32, tag="st", name="stats")

                si = 0  # bn_stats slot index within sample
                for xr0, nxr, or0, nor in CHUNKS:
                    xt = xpool.tile([128, XROWS_MAX, W], f32r, tag="x", name="xt")
                    nc.sync.dma_start(
                        xt[0:64, 0:nxr, :], xs[b, :, xr0 : xr0 + nxr, :]
                    )
                    n1 = min(nxr, H - 1 - xr0)
                    nc.sync.dma_start(
                        xt[64:128, 0:n1, :], xs[b, :, xr0 + 1 : xr0 + 1 + n1, :]
                    )

                    g0 = or0
                    while g0 < or0 + nor:
                        gn = min(8, or0 + nor - g0)  # 8 or 6 output rows
                        hr = gn // 2  # rows per half (4 or 3)
                        cp = cps.tile([128, 1024], f32, tag="cp", name="cp")
                        for half in range(2):
                            row0 = g0 + half * hr
                            l0 = row0 - xr0
                            outap = cp[:, half * 512 : half * 512 + hr * OW]
                            for kw in range(3):
                                nc.tensor.matmul(
                                    outap,
                                    wp[:, kw * COUT : (kw + 1) * COUT],
                                    xt[:, l0 : l0 + hr, kw : kw + OW],
                                    start=(kw == 0),
                                    stop=False,
                                )
                            for kw in range(3):
                                nc.tensor.matmul(
                                    outap,
                                    ws[:, kw * COUT : (kw + 1) * COUT],
                                    xt[0:64, l0 + 2 : l0 + 2 + hr, kw : kw + OW],
                                    start=False,
                                    stop=(kw == 2),
                                )
                        # evacuate both halves in one strided ACT copy
                        yv = y_raw[:, g0 * OW : (g0 + gn) * OW].rearrange(
                            "p (a b) -> p a b", b=hr * OW
                        )
                        nc.scalar.activation(
                            yv,
                            cp[:].rearrange("p (a b) -> p a b", b=512)[
                                :, :, 0 : hr * OW
                            ],
                            AF.Copy,
                        )
                        # one-pass stats on the freshly evacuated rows
                        # (bn_stats free size is capped at 512 -> one call
                        # per half-group of hr*126 <= 504 elements)
                        for half in range(2):
                            r0 = (g0 + half * hr) * OW
                            nc.vector.bn_stats(
                                stats[:, si, :],
                                y_raw[:, r0 : r0 + hr * OW],
                            )
                            si += 1
                        g0 += gn

                # aggregate per-channel mean/var, then A/B affine coefficients
                mv = stpool.tile([128, 2], f32, tag="mv", name="mv")
                nc.vector.bn_aggr(mv[:], stats[:])
                st = stpool.tile([128, 2], f32, tag="sts", name="st")
                # t1 = mean + conv_b ; t2 = var + t1^2
                nc.vector.tensor_tensor(
                    st[:, 0:1], mv[:, 0:1], cb[:], OP.add
                )
                t1sq = stpool.tile([128, 1], f32, tag="t1sq", name="t1sq")
                nc.vector.tensor_tensor(t1sq[:], st[:, 0:1], st[:, 0:1], OP.mult)
                nc.vector.tensor_tensor(st[:, 1:2], mv[:, 1:2], t1sq[:], OP.add)
                gsum = gps.tile([128, 2], f32, tag="gsum", name="gsum")
                nc.tensor.matmul(gsum[:], bones[:], st[:], start=True, stop=True)
                # m = gsum0/8 ; ex2 = gsum1/8 ; v = ex2 - m*m
                mgrp = stpool.tile([128, 1], f32, tag="mgrp", name="mgrp")
                nc.vector.tensor_scalar(
                    mgrp[:], gsum[:, 0:1], 1.0 / GSZ, None, OP.mult
                )
                vgrp = stpool.tile([128, 1], f32, tag="vgrp", name="vgrp")
                nc.vector.tensor_scalar(
                    vgrp[:], gsum[:, 1:2], 1.0 / GSZ, EPS, OP.mult, OP.add
                )
                msq = stpool.tile([128, 1], f32, tag="msq", name="msq")
                nc.vector.tensor_tensor(msq[:], mgrp[:], mgrp[:], OP.mult)
                nc.vector.tensor_tensor(vgrp[:], vgrp[:], msq[:], OP.subtract)
                # inv = 1/sqrt(v+eps) ; A = inv*gs ; B = (cb-m)*A + gbs
                sdev = stpool.tile([128, 1], f32, tag="sdev", name="sdev")
                nc.scalar.activation(sdev[:], vgrp[:], AF.Sqrt, bias=zeros1[:])
                inv = stpool.tile([128, 1], f32, tag="inv", name="inv")
                nc.vector.reciprocal(inv[:], sdev[:])
                Acoef = stpool.tile([128, 1], f32, tag="Ac", name="Acoef")
                nc.vector.tensor_tensor(Acoef[:], inv[:], gs[:], OP.mult)
                Bcoef = stpool.tile([128, 1], f32, tag="Bc", name="Bcoef")
                nc.vector.tensor_tensor(Bcoef[:], cb[:], mgrp[:], OP.subtract)
                nc.vector.tensor_tensor(Bcoef[:], Bcoef[:], Acoef[:], OP.mult)
                nc.vector.tensor_tensor(Bcoef[:], Bcoef[:], gbs[:], OP.add)

                # affine + relu in place (4 chunks), then pool + clamp
                ph = phpool.tile([128, OH, PW], f32, tag="ph", name="ph")
                y3 = y_raw[:].rearrange("p (a b) -> p a b", b=OW)
                for q in range(4):
                    r0 = q * 32
                    r1 = min(OH, r0 + 32)
                    seg = y_raw[:, r0 * OW : r1 * OW]
                    nc.scalar.activation(
                        seg, seg, AF.Relu, bias=Bcoef[:], scale=Acoef[:]
                    )
                    nc.vector.tensor_tensor(
                        ph[:, r0:r1, :],
                        y3[:, r0:r1, 0 : OW : 2],
                        y3[:, r0:r1, 1 : OW : 2],
                        OP.max,
                    )
                pv = pvpool.tile([128, PH, PW], f32, tag="pv", name="pv")
                nc.vector.tensor_tensor(
                    pv[:], ph[:, 0 : OH : 2, :], ph[:, 1 : OH : 2, :], OP.max
                )
                nc.vector.tensor_scalar(
                    pv[:], pv[:], 1.0, None, OP.min
                )
                nc.sync.dma_start(
                    out_d[b].rearrange("c h w -> c (h w)"),
                    pv[:].rearrange("p a b -> p (a b)"),
                )

    nc.finalize()
    _CACHED["nc"] = nc
    return nc


def _prep_consts(conv_w, conv_b, gn_w, gn_b, scale):
    # wp[kw, ci + 64*kh, co] = conv_w[co, ci, kh, kw] for kh in {0,1}
    w = np.ascontiguousarray(conv_w.astype(np.float32))
    wp = np.empty((3, 128, COUT), np.float32)
    ws = np.empty((3, 64, COUT), np.float32)
    for kw in range(3):
        wp[kw, 0:64, :] = w[:, :, 0, kw].T
        wp[kw, 64:128, :] = w[:, :, 1, kw].T
        ws[kw, :, :] = w[:, :, 2, kw].T
    cb = conv_b.astype(np.float32).reshape(COUT, 1)
    sc = scale.astype(np.float32).reshape(COUT)
    gs = (gn_w.astype(np.float32) * sc).reshape(COUT, 1)
    gbs = (gn_b.astype(np.float32) * sc).reshape(COUT, 1)
    bones = np.zeros((COUT, COUT), np.float32)
    for g in range(NG):
        bones[g * GSZ : (g + 1) * GSZ, g * GSZ : (g + 1) * GSZ] = 1.0
    return wp, ws, cb, gs, gbs, bones


def kernel(x, conv_w, conv_b, gn_w, gn_b, scale):
    x = np.ascontiguousarray(np.asarray(x, dtype=np.float32))
    wp, ws, cb, gs, gbs, bones = _prep_consts(
        np.asarray(conv_w), np.asarray(conv_b), np.asarray(gn_w),
        np.asarray(gn_b), np.asarray(scale),
    )
    nc = _build()
    in_maps = []
    for c in range(N_CORES):
        in_maps.append({
            "xs": x[c * BPC : (c + 1) * BPC],
            "wp": wp, "ws": ws, "cb": cb, "gs": gs, "gbs": gbs,
            "bones": bones,
        })
    res = run_bass_kernel_spmd(nc, in_maps, core_ids=list(range(N_CORES)))
    out = np.concatenate([res.results[c]["out"] for c in range(N_CORES)], axis=0)
    return out.astype(np.float32)


if __name__ == "__main__":
    rng = np.random.default_rng(0)
    x = rng.standard_normal((B_FULL, CIN, H, W), dtype=np.float32)
    cw = rng.standard_normal((COUT, CIN, 3, 3), dtype=np.float32)
    out = kernel(x, cw, rng.standard_normal(COUT, dtype=np.float32),
                 rng.standard_normal(COUT, dtype=np.float32),
                 rng.standard_normal(COUT, dtype=np.float32),
                 rng.standard_normal((COUT, 1, 1), dtype=np.float32))
    print(out.shape, out.dtype)


# revision 10
# speedup vs baseline: 1.0782x; 1.0782x over previous
"""Fused conv3x3 -> GroupNorm(16) -> channel scale -> maxpool2x2 -> clamp[0,1]
Trainium2 Bass kernel, data-parallel over batch on 8 NeuronCores.

Input  x [32, 64, 128, 128] f32  -> output [32, 128, 63, 63] f32.
Each core handles 4 samples.

Conv strategy: tap-wise matmuls with contraction over (cin, kh-pair) on the
128 SBUF partitions. SBUF x-buffer layout: partition p = ci + 64*r holds
x[ci, row+r, :] (r in {0,1}), so one matmul with a [128, 128] stacked weight
covers taps (kh=0, kw) and (kh=1, kw) at once; kh=2 taps run as 64-partition
matmuls. 6 matmuls per 4 output rows, fp32r (full PE rate, ~1e-4 rel err).

GroupNorm: per-channel mean/var via one-pass bn_stats/bn_aggr on the DVE,
8-channel group reduction via a tiny block-diagonal-ones matmul (fp32),
conv bias folded analytically into the final per-channel affine
z = A*y + B, which is applied on the ScalarE with fused Relu (lower clamp).
Maxpool as two strided tensor_tensor max ops; upper clamp fused into a
tensor_scalar on the pooled tile.
"""

import numpy as np

import concourse.bacc as bacc
import concourse.mybir as mybir
import concourse.tile as tile
from concourse.bass_utils import run_bass_kernel_spmd

N_CORES = 8
B_FULL, CIN, H, W = 32, 64, 128, 128
COUT = 128
BPC = B_FULL // N_CORES  # samples per core
OH = OW = 126
PH = PW = 63
NG = 16  # groups
GSZ = COUT // NG  # 8 channels per group
EPS = 1e-5
S = OH * OW  # spatial size per sample

# (x_row0, n_xrows, out_row0, n_out_rows)
CHUNKS = [(0, 66, 0, 64), (64, 64, 64, 62)]
XROWS_MAX = 66

_CACHED = {}


def _build():
    if "nc" in _CACHED:
        return _CACHED["nc"]
    f32 = mybir.dt.float32
    f32r = mybir.dt.float32r
    AF = mybir.ActivationFunctionType
    OP = mybir.AluOpType

    nc = bacc.Bacc("TRN2", target_bir_lowering=False, debug=False)
    xs = nc.dram_tensor("xs", [BPC, CIN, H, W], f32r, kind="ExternalInput").ap()
    wp_d = nc.dram_tensor("wp", [3, 128, COUT], f32r, kind="ExternalInput").ap()
    ws_d = nc.dram_tensor("ws", [3, 64, COUT], f32r, kind="ExternalInput").ap()
    cb_d = nc.dram_tensor("cb", [COUT, 1], f32, kind="ExternalInput").ap()
    gs_d = nc.dram_tensor("gs", [COUT, 1], f32, kind="ExternalInput").ap()
    gbs_d = nc.dram_tensor("gbs", [COUT, 1], f32, kind="ExternalInput").ap()
    bones_d = nc.dram_tensor("bones", [COUT, COUT], f32, kind="ExternalInput").ap()
    out_d = nc.dram_tensor("out", [BPC, COUT, PH, PW], f32, kind="ExternalOutput").ap()

    with tile.TileContext(nc) as tc:
        with (
            tc.tile_pool(name="consts", bufs=1) as cpool,
            tc.tile_pool(name="xpool", bufs=2) as xpool,
            tc.tile_pool(name="ypool", bufs=1) as ypool,
            tc.tile_pool(name="stpool", bufs=2) as stpool,
            tc.tile_pool(name="phpool", bufs=1) as phpool,
            tc.tile_pool(name="pvpool", bufs=2) as pvpool,
            tc.tile_pool(name="cps", bufs=3, space="PSUM") as cps,
            tc.tile_pool(name="gps", bufs=1, space="PSUM") as gps,
        ):
            wp = cpool.tile([128, 3 * COUT], f32r, name="wp_t")
            ws = cpool.tile([64, 3 * COUT], f32r, name="ws_t")
            for kw in range(3):
                nc.sync.dma_start(wp[:, kw * COUT : (kw + 1) * COUT], wp_d[kw])
                nc.sync.dma_start(ws[:, kw * COUT : (kw + 1) * COUT], ws_d[kw])
            cb = cpool.tile([COUT, 1], f32, name="cb_t")
            nc.sync.dma_start(cb[:], cb_d[:])
            gs = cpool.tile([COUT, 1], f32, name="gs_t")
            nc.sync.dma_start(gs[:], gs_d[:])
            gbs = cpool.tile([COUT, 1], f32, name="gbs_t")
            nc.sync.dma_start(gbs[:], gbs_d[:])
            bones = cpool.tile([COUT, COUT], f32, name="bones_t")
            nc.sync.dma_start(bones[:], bones_d[:])
            zeros1 = cpool.tile([COUT, 1], f32, name="zeros1")
            nc.vector.memset(zeros1[:], 0.0)

            for b in range(BPC):
                y_raw = ypool.tile([128, S], f32, tag="y", name="y_raw")
                stats = stpool.tile([128, 32, 6], f32, tag="st", name="stats")

                si = 0  # bn_stats slot index within sample
                for xr0, nxr, or0, nor in CHUNKS:
                    xt = xpool.tile([128, XROWS_MAX, W], f32r, tag="x", name="xt")
                    nc.sync.dma_start(
                        xt[0:64, 0:nxr, :], xs[b, :, xr0 : xr0 + nxr, :]
                    )
                    n1 = min(nxr, H - 1 - xr0)
                    nc.sync.dma_start(
                        xt[64:128, 0:n1, :], xs[b, :, xr0 + 1 : xr0 + 1 + n1, :]
                    )

                    g0 = or0
                    while g0 < or0 + nor:
                        gn = min(8, or0 + nor - g0)  # 8 or 6 output rows
                        hr = gn // 2  # rows per half (4 or 3)
                        cp = cps.tile([128, 1024], f32, tag="cp", name="cp")
                        for half in range(2):
                            row0 = g0 + half * hr
                            l0 = row0 - xr0
                            outap = cp[:, half * 512 : half * 512 + hr * OW]
                            for kw in range(3):
                                nc.tensor.matmul(
                                    outap,
                                    wp[:, kw * COUT : (kw + 1) * COUT],
                                    xt[:, l0 : l0 + hr, kw : kw + OW],
                                    start=(kw == 0),
                                    stop=False,
                                )
                            for kw in range(3):
                                nc.tensor.matmul(
                                    outap,
                                    ws[:, kw * COUT : (kw + 1) * COUT],
                                    xt[0:64, l0 + 2 : l0 + 2 + hr, kw : kw + OW],
                                    start=False,
                                    stop=(kw == 2),
                                )
                        # evacuate both halves in one strided ACT copy
                        yv = y_raw[:, g0 * OW : (g0 + gn) * OW].rearrange(
                            "p (a b) -> p a b", b=hr * OW
                        )
                        nc.scalar.activation(
                            yv,
                            cp[:].rearrange("p (a b) -> p a b", b=512)[
                                :, :, 0 : hr * OW
                            ],
                            AF.Copy,
                        )
                        # one-pass stats on the freshly evacuated rows
                        # (bn_stats free size is capped at 512 -> one call
                        # per half-group of hr*126 <= 504 elements)
                        for half in range(2):
                            r0 = (g0 + half * hr) * OW
                            nc.vector.bn_stats(
                                stats[:, si, :],
                                y_raw[:, r0 : r0 + hr * OW],
                            )
                            si += 1
                        g0 += gn

                # aggregate per-channel mean/var, then A/B affine coefficients
                mv = stpool.tile([128, 2], f32, tag="mv", name="mv")
                nc.vector.bn_aggr(mv[:], stats[:])
                st = stpool.tile([128, 2], f32, tag="sts", name="st")
                # t1 = mean + conv_b ; t2 = var + t1^2
                nc.vector.tensor_tensor(
                    st[:, 0:1], mv[:, 0:1], cb[:], OP.add
                )
                t1sq = stpool.tile([128, 1], f32, tag="t1sq", name="t1sq")
                nc.vector.tensor_tensor(t1sq[:], st[:, 0:1], st[:, 0:1], OP.mult)
                nc.vector.tensor_tensor(st[:, 1:2], mv[:, 1:2], t1sq[:], OP.add)
                gsum = gps.tile([128, 2], f32, tag="gsum", name="gsum")
                nc.tensor.matmul(gsum[:], bones[:], st[:], start=True, stop=True)
                # m = gsum0/8 ; ex2 = gsum1/8 ; v = ex2 - m*m
                mgrp = stpool.tile([128, 1], f32, tag="mgrp", name="mgrp")
                nc.vector.tensor_scalar(
                    mgrp[:], gsum[:, 0:1], 1.0 / GSZ, None, OP.mult
                )
                vgrp = stpool.tile([128, 1], f32, tag="vgrp", name="vgrp")
                nc.vector.tensor_scalar(
                    vgrp[:], gsum[:, 1:2], 1.0 / GSZ, EPS, OP.mult, OP.add
                )
                msq = stpool.tile([128, 1], f32, tag="msq", name="msq")
                nc.vector.tensor_tensor(msq[:], mgrp[:], mgrp[:], OP.mult)
                nc.vector.tensor_tensor(vgrp[:], vgrp[:], msq[:], OP.subtract)
                # inv = 1/sqrt(v+eps) ; A = inv*gs ; B = (cb-m)*A + gbs
                sdev = stpool.tile([128, 1], f32, tag="sdev", name="sdev")
                nc.scalar.activation(sdev[:], vgrp[:], AF.Sqrt, bias=zeros1[:])
                inv = stpool.tile([128, 1], f32, tag="inv", name="inv")
                nc.vector.reciprocal(inv[:], sdev[:])
                Acoef = stpool.tile([128, 1], f32, tag="Ac", name="Acoef")
                nc.vector.tensor_tensor(Acoef[:], inv[:], gs[:], OP.mult)
                Bcoef = stpool.tile([128, 1], f32, tag="Bc", name="Bcoef")
                nc.vector.tensor_tensor(Bcoef[:], cb[:], mgrp[:], OP.subtract)
                nc.vector.tensor_tensor(Bcoef[:], Bcoef[:], Acoef[:], OP.mult)
                nc.vector.tensor_tensor(Bcoef[:], Bcoef[:], gbs[:], OP.add)

                # affine + relu in place (4 chunks), then pool + clamp
                ph = phpool.tile([128, OH, PW], f32, tag="ph", name="ph")
                y3 = y_raw[:].rearrange("p (a b) -> p a b", b=OW)
                for q in range(4):
                    r0 = q * 32
                    r1 = min(OH, r0 + 32)
                    seg = y_raw[:, r0 * OW : r1 * OW]
                    nc.scalar.activation(
                        seg, seg, AF.Relu, bias=Bcoef[:], scale=Acoef[:]
                    )
                    nc.vector.tensor_tensor(
                        ph[:, r0:r1, :],
                        y3[:, r0:r1, 0 : OW : 2],
                        y3[:, r0:r1, 1 : OW : 2],
                        OP.max,
                    )
                pv = pvpool.tile([128, PH, PW], f32, tag="pv", name="pv")
                nc.vector.tensor_tensor(
                    pv[:], ph[:, 0 : OH : 2, :], ph[:, 1 : OH : 2, :], OP.max
                )
                nc.vector.tensor_scalar(
                    pv[:], pv[:], 1.0, None, OP.min
                )
                nc.sync.dma_start(
                    out_d[b].rearrange("c h w -> c (h w)"),
                    pv[:].rearrange("p a b -> p (a b)"),
                )

    nc.finalize()
    _CACHED["nc"] = nc
    return nc


def _prep_consts(conv_w, conv_b, gn_w, gn_b, scale):
    # wp[kw, ci + 64*kh, co] = conv_w[co, ci, kh, kw] for kh in {0,1}
    w = np.ascontiguousarray(conv_w.astype(np.float32))
    wp = np.empty((3, 128, COUT), np.float32)
    ws = np.empty((3, 64, COUT), np.float32)
    for kw in range(3):
        wp[kw, 0:64, :] = w[:, :, 0, kw].T
        wp[kw, 64:128, :] = w[:, :, 1, kw].T
        ws[kw, :, :] = w[:, :, 2, kw].T
    cb = conv_b.astype(np.float32).reshape(COUT, 1)
    sc = scale.astype(np.float32).reshape(COUT)
    gs = (gn_w.astype(np.float32) * sc).reshape(COUT, 1)
    gbs = (gn_b.astype(np.float32) * sc).reshape(COUT, 1)
    bones = np.zeros((COUT, COUT), np.float32)
    for g in range(NG):
        bones[g * GSZ : (g + 1) * GSZ, g * GSZ : (g + 1) * GSZ] = 1.0
    return wp, ws, cb, gs, gbs, bones


def kernel(x, conv_w, conv_b, gn_w, gn_b, scale):
    x = np.ascontiguousarray(np.asarray(x, dtype=np.float32))
    wp, ws, cb, gs, gbs, bones = _prep_consts(
        np.asarray(conv_w), np.asarray(conv_b), np.asarray(gn_w),
        np.asarray(gn_b), np.asarray(scale),
    )
    nc = _build()
    in_maps = []
    for c in range(N_CORES):
        in_maps.append({
            "xs": x[c * BPC : (c + 1) * BPC],
            "wp": wp, "ws": ws, "cb": cb, "gs": gs, "gbs": gbs,
            "bones": bones,
        })
    results = _run_cached(nc, in_maps)
    out = np.concatenate([results[c]["out"] for c in range(N_CORES)], axis=0)
    return out.astype(np.float32)


def _run_cached(nc, in_maps):
    """run_bass_kernel_spmd's axon path with the jitted executable cached
    across calls (avoids re-tracing the shard_map wrapper every call)."""
    import jax
    import numpy as _np
    from jax.sharding import Mesh, PartitionSpec
    from jax.experimental.shard_map import shard_map
    from concourse import bass2jax

    if "runner" not in _CACHED:
        bass2jax.install_neuronx_cc_hook()
        partition_name = (
            nc.partition_id_tensor.name if nc.partition_id_tensor else None
        )
        in_names, out_names, out_avals, zero_outs = [], [], [], []
        for alloc in nc.m.functions[0].allocations:
            if not isinstance(alloc, mybir.MemoryLocationSet):
                continue
            name = alloc.memorylocations[0].name
            if alloc.kind == "ExternalInput":
                if name != partition_name:
                    in_names.append(name)
            elif alloc.kind == "ExternalOutput":
                shape = tuple(alloc.tensor_shape)
                dtype = mybir.dt.np(alloc.dtype)
                out_names.append(name)
                out_avals.append(jax.core.ShapedArray(shape, dtype))
                zero_outs.append(_np.zeros(shape, dtype))
        n_params = len(in_names)
        n_outs = len(out_avals)
        all_names = list(in_names) + list(out_names)
        if partition_name is not None:
            all_names.append(partition_name)
        donate = tuple(range(n_params, n_params + n_outs))

        def _body(*args):
            operands = list(args)
            if partition_name is not None:
                operands.append(bass2jax.partition_id_tensor())
            outs = bass2jax._bass_exec_p.bind(
                *operands,
                out_avals=tuple(out_avals),
                in_names=tuple(all_names),
                out_names=tuple(out_names),
                lowering_input_output_aliases=(),
                sim_require_finite=True,
                sim_require_nnan=True,
                nc=nc,
            )
            return tuple(outs)

        devices = jax.devices()[:N_CORES]
        mesh = Mesh(_np.asarray(devices), ("core",))
        in_specs = (PartitionSpec("core"),) * (n_params + n_outs)
        out_specs = (PartitionSpec("core"),) * n_outs
        sharded = jax.jit(
            shard_map(_body, mesh=mesh, in_specs=in_specs,
                      out_specs=out_specs, check_rep=False),
            donate_argnums=donate, keep_unused=True,
        )
        _CACHED["runner"] = (sharded, in_names, out_names, out_avals, zero_outs)

    sharded, in_names, out_names, out_avals, zero_outs = _CACHED["runner"]
    import numpy as _np2
    concat_in = [
        _np2.concatenate([_np2.asarray(in_maps[c][n]) for c in range(N_CORES)], axis=0)
        for n in in_names
    ]
    concat_zeros = [
        _np2.zeros((N_CORES * z.shape[0], *z.shape[1:]), z.dtype) for z in zero_outs
    ]
    out_arrs = sharded(*concat_in, *concat_zeros)
    return [
        {
            name: _np2.asarray(out_arrs[i]).reshape(N_CORES, *out_avals[i].shape)[c]
            for i, name in enumerate(out_names)
        }
        for c in range(N_CORES)
    ]


if __name__ == "__main__":
    rng = np.random.default_rng(0)
    x = rng.standard_normal((B_FULL, CIN, H, W), dtype=np.float32)
    cw = rng.standard_normal((COUT, CIN, 3, 3), dtype=np.float32)
    out = kernel(x, cw, rng.standard_normal(COUT, dtype=np.float32),
                 rng.standard_normal(COUT, dtype=np.float32),
                 rng.standard_normal(COUT, dtype=np.float32),
                 rng.standard_normal((COUT, 1, 1), dtype=np.float32))
    print(out.shape, out.dtype)


# revision 12
# speedup vs baseline: 7480.6878x; 6938.1305x over previous
"""Fused conv3x3 -> GroupNorm(16) -> channel scale -> maxpool2x2 -> clamp[0,1]
Trainium2 Bass kernel, data-parallel over batch on 8 NeuronCores.

Input  x [32, 64, 128, 128] f32  -> output [32, 128, 63, 63] f32.
Each core handles 4 samples.

Conv strategy: tap-wise matmuls with contraction over (cin, kh-pair) on the
128 SBUF partitions. SBUF x-buffer layout: partition p = ci + 64*r holds
x[ci, row+r, :] (r in {0,1}), so one matmul with a [128, 128] stacked weight
covers taps (kh=0, kw) and (kh=1, kw) at once; kh=2 taps run as 64-partition
matmuls. 6 matmuls per 4 output rows, fp32r (full PE rate, ~1e-4 rel err).

GroupNorm: per-channel mean/var via one-pass bn_stats/bn_aggr on the DVE,
8-channel group reduction via a tiny block-diagonal-ones matmul (fp32),
conv bias folded analytically into the final per-channel affine
z = A*y + B, which is applied on the ScalarE with fused Relu (lower clamp).
Maxpool as two strided tensor_tensor max ops; upper clamp fused into a
tensor_scalar on the pooled tile.
"""

import numpy as np

import concourse.bacc as bacc
import concourse.mybir as mybir
import concourse.tile as tile
from concourse.bass_utils import run_bass_kernel_spmd

N_CORES = 8
B_FULL, CIN, H, W = 32, 64, 128, 128
COUT = 128
BPC = B_FULL // N_CORES  # samples per core
OH = OW = 126
PH = PW = 63
NG = 16  # groups
GSZ = COUT // NG  # 8 channels per group
EPS = 1e-5
S = OH * OW  # spatial size per sample

# (x_row0, n_xrows, out_row0, n_out_rows)
CHUNKS = [(0, 66, 0, 64), (64, 64, 64, 62)]
XROWS_MAX = 66

_CACHED = {}


def _build():
    if "nc" in _CACHED:
        return _CACHED["nc"]
    f32 = mybir.dt.float32
    f32r = mybir.dt.float32r
    bf16 = mybir.dt.bfloat16
    AF = mybir.ActivationFunctionType
    OP = mybir.AluOpType

    nc = bacc.Bacc("TRN2", target_bir_lowering=False, debug=False)
    xs = nc.dram_tensor("xs", [BPC, CIN, H, W], f32r, kind="ExternalInput").ap()
    wp_d = nc.dram_tensor("wp", [3, 128, COUT], f32r, kind="ExternalInput").ap()
    ws_d = nc.dram_tensor("ws", [3, 64, COUT], f32r, kind="ExternalInput").ap()
    cb_d = nc.dram_tensor("cb", [COUT, 1], f32, kind="ExternalInput").ap()
    gs_d = nc.dram_tensor("gs", [COUT, 1], f32, kind="ExternalInput").ap()
    gbs_d = nc.dram_tensor("gbs", [COUT, 1], f32, kind="ExternalInput").ap()
    bones_d = nc.dram_tensor("bones", [COUT, COUT], f32, kind="ExternalInput").ap()
    out_d = nc.dram_tensor("out", [BPC, COUT, PH, PW], f32, kind="ExternalOutput").ap()

    with tile.TileContext(nc) as tc:
        with (
            tc.tile_pool(name="consts", bufs=1) as cpool,
            tc.tile_pool(name="xpool", bufs=2) as xpool,
            tc.tile_pool(name="ypool", bufs=1) as ypool,
            tc.tile_pool(name="stpool", bufs=2) as stpool,
            tc.tile_pool(name="phpool", bufs=1) as phpool,
            tc.tile_pool(name="pvpool", bufs=2) as pvpool,
            tc.tile_pool(name="cps", bufs=3, space="PSUM") as cps,
            tc.tile_pool(name="gps", bufs=1, space="PSUM") as gps,
        ):
            wp = cpool.tile([128, 3 * COUT], f32r, name="wp_t")
            ws = cpool.tile([64, 3 * COUT], f32r, name="ws_t")
            for kw in range(3):
                nc.sync.dma_start(wp[:, kw * COUT : (kw + 1) * COUT], wp_d[kw])
                nc.sync.dma_start(ws[:, kw * COUT : (kw + 1) * COUT], ws_d[kw])
            cb = cpool.tile([COUT, 1], f32, name="cb_t")
            nc.sync.dma_start(cb[:], cb_d[:])
            gs = cpool.tile([COUT, 1], f32, name="gs_t")
            nc.sync.dma_start(gs[:], gs_d[:])
            gbs = cpool.tile([COUT, 1], f32, name="gbs_t")
            nc.sync.dma_start(gbs[:], gbs_d[:])
            bones = cpool.tile([COUT, COUT], f32, name="bones_t")
            nc.sync.dma_start(bones[:], bones_d[:])
            zeros1 = cpool.tile([COUT, 1], f32, name="zeros1")
            nc.vector.memset(zeros1[:], 0.0)

            for b in range(BPC):
                y_raw = ypool.tile([128, S], f32, tag="y", name="y_raw")
                stats = stpool.tile([128, 32, 6], f32, tag="st", name="stats")

                si = 0  # bn_stats slot index within sample
                for xr0, nxr, or0, nor in CHUNKS:
                    xt = xpool.tile([128, XROWS_MAX, W], f32r, tag="x", name="xt")
                    nc.sync.dma_start(
                        xt[0:64, 0:nxr, :], xs[b, :, xr0 : xr0 + nxr, :]
                    )
                    n1 = min(nxr, H - 1 - xr0)
                    nc.sync.dma_start(
                        xt[64:128, 0:n1, :], xs[b, :, xr0 + 1 : xr0 + 1 + n1, :]
                    )

                    g0 = or0
                    while g0 < or0 + nor:
                        gn = min(8, or0 + nor - g0)  # 8 or 6 output rows
                        hr = gn // 2  # rows per half (4 or 3)
                        cp = cps.tile([128, 1024], f32, tag="cp", name="cp")
                        for half in range(2):
                            row0 = g0 + half * hr
                            l0 = row0 - xr0
                            outap = cp[:, half * 512 : half * 512 + hr * OW]
                            for kw in range(3):
                                nc.tensor.matmul(
                                    outap,
                                    wp[:, kw * COUT : (kw + 1) * COUT],
                                    xt[:, l0 : l0 + hr, kw : kw + OW],
                                    start=(kw == 0),
                                    stop=False,
                                )
                            for kw in range(3):
                                nc.tensor.matmul(
                                    outap,
                                    ws[:, kw * COUT : (kw + 1) * COUT],
                                    xt[0:64, l0 + 2 : l0 + 2 + hr, kw : kw + OW],
                                    start=False,
                                    stop=(kw == 2),
                                )
                        # evacuate both halves in one strided ACT copy
                        yv = y_raw[:, g0 * OW : (g0 + gn) * OW].rearrange(
                            "p (a b) -> p a b", b=hr * OW
                        )
                        nc.scalar.activation(
                            yv,
                            cp[:].rearrange("p (a b) -> p a b", b=512)[
                                :, :, 0 : hr * OW
                            ],
                            AF.Copy,
                        )
                        # one-pass stats on the freshly evacuated rows
                        # (bn_stats free size is capped at 512 -> one call
                        # per half-group of hr*126 <= 504 elements)
                        for half in range(2):
                            r0 = (g0 + half * hr) * OW
                            nc.vector.bn_stats(
                                stats[:, si, :],
                                y_raw[:, r0 : r0 + hr * OW],
                            )
                            si += 1
                        g0 += gn

                # aggregate per-channel mean/var, then A/B affine coefficients
                mv = stpool.tile([128, 2], f32, tag="mv", name="mv")
                nc.vector.bn_aggr(mv[:], stats[:])
                st = stpool.tile([128, 2], f32, tag="sts", name="st")
                # t1 = mean + conv_b ; t2 = var + t1^2
                nc.vector.tensor_tensor(
                    st[:, 0:1], mv[:, 0:1], cb[:], OP.add
                )
                t1sq = stpool.tile([128, 1], f32, tag="t1sq", name="t1sq")
                nc.vector.tensor_tensor(t1sq[:], st[:, 0:1], st[:, 0:1], OP.mult)
                nc.vector.tensor_tensor(st[:, 1:2], mv[:, 1:2], t1sq[:], OP.add)
                gsum = gps.tile([128, 2], f32, tag="gsum", name="gsum")
                nc.tensor.matmul(gsum[:], bones[:], st[:], start=True, stop=True)
                # m = gsum0/8 ; ex2 = gsum1/8 ; v = ex2 - m*m
                mgrp = stpool.tile([128, 1], f32, tag="mgrp", name="mgrp")
                nc.vector.tensor_scalar(
                    mgrp[:], gsum[:, 0:1], 1.0 / GSZ, None, OP.mult
                )
                vgrp = stpool.tile([128, 1], f32, tag="vgrp", name="vgrp")
                nc.vector.tensor_scalar(
                    vgrp[:], gsum[:, 1:2], 1.0 / GSZ, EPS, OP.mult, OP.add
                )
                msq = stpool.tile([128, 1], f32, tag="msq", name="msq")
                nc.vector.tensor_tensor(msq[:], mgrp[:], mgrp[:], OP.mult)
                nc.vector.tensor_tensor(vgrp[:], vgrp[:], msq[:], OP.subtract)
                # inv = 1/sqrt(v+eps) ; A = inv*gs ; B = (cb-m)*A + gbs
                sdev = stpool.tile([128, 1], f32, tag="sdev", name="sdev")
                nc.scalar.activation(sdev[:], vgrp[:], AF.Sqrt, bias=zeros1[:])
                inv = stpool.tile([128, 1], f32, tag="inv", name="inv")
                nc.vector.reciprocal(inv[:], sdev[:])
                Acoef = stpool.tile([128, 1], f32, tag="Ac", name="Acoef")
                nc.vector.tensor_tensor(Acoef[:], inv[:], gs[:], OP.mult)
                Bcoef = stpool.tile([128, 1], f32, tag="Bc", name="Bcoef")
                nc.vector.tensor_tensor(Bcoef[:], cb[:], mgrp[:], OP.subtract)
                nc.vector.tensor_tensor(Bcoef[:], Bcoef[:], Acoef[:], OP.mult)
                nc.vector.tensor_tensor(Bcoef[:], Bcoef[:], gbs[:], OP.add)

                # affine + relu in place (4 chunks), then pool + clamp
                ph = phpool.tile([128, OH, PW], f32, tag="ph", name="ph")
                y3 = y_raw[:].rearrange("p (a b) -> p a b", b=OW)
                for q in range(4):
                    r0 = q * 32
                    r1 = min(OH, r0 + 32)
                    seg = y_raw[:, r0 * OW : r1 * OW]
                    nc.scalar.activation(
                        seg, seg, AF.Relu, bias=Bcoef[:], scale=Acoef[:]
                    )
                    nc.vector.tensor_tensor(
                        ph[:, r0:r1, :],
                        y3[:, r0:r1, 0 : OW : 2],
                        y3[:, r0:r1, 1 : OW : 2],
                        OP.max,
                    )
                pv = pvpool.tile([128, PH, PW], f32, tag="pv", name="pv")
                nc.vector.tensor_tensor(
                    pv[:], ph[:, 0 : OH : 2, :], ph[:, 1 : OH : 2, :], OP.max
                )
                nc.vector.tensor_scalar(
                    pv[:], pv[:], 1.0, None, OP.min
                )
                nc.sync.dma_start(
                    out_d[b].rearrange("c h w -> c (h w)"),
                    pv[:].rearrange("p a b -> p (a b)"),
                )

    nc.finalize()
    _CACHED["nc"] = nc
    return nc


def _prep_consts(conv_w, conv_b, gn_w, gn_b, scale):
    # wp[kw, ci + 64*kh, co] = conv_w[co, ci, kh, kw] for kh in {0,1}
    w = np.ascontiguousarray(conv_w.astype(np.float32))
    wp = np.empty((3, 128, COUT), np.float32)
    ws = np.empty((3, 64, COUT), np.float32)
    for kw in range(3):
        wp[kw, 0:64, :] = w[:, :, 0, kw].T
        wp[kw, 64:128, :] = w[:, :, 1, kw].T
        ws[kw, :, :] = w[:, :, 2, kw].T
    cb = conv_b.astype(np.float32).reshape(COUT, 1)
    sc = scale.astype(np.float32).reshape(COUT)
    gs = (gn_w.astype(np.float32) * sc).reshape(COUT, 1)
    gbs = (gn_b.astype(np.float32) * sc).reshape(COUT, 1)
    bones = np.zeros((COUT, COUT), np.float32)
    for g in range(NG):
        bones[g * GSZ : (g + 1) * GSZ, g * GSZ : (g + 1) * GSZ] = 1.0
    return wp, ws, cb, gs, gbs, bones


def kernel(x, conv_w, conv_b, gn_w, gn_b, scale):
    x = np.ascontiguousarray(np.asarray(x, dtype=np.float32))
    wp, ws, cb, gs, gbs, bones = _prep_consts(
        np.asarray(conv_w), np.asarray(conv_b), np.asarray(gn_w),
        np.asarray(gn_b), np.asarray(scale),
    )
    nc = _build()
    in_maps = []
    for c in range(N_CORES):
        in_maps.append({
            "xs": x[c * BPC : (c + 1) * BPC],
            "wp": wp, "ws": ws,
            "cb": cb, "gs": gs, "gbs": gbs, "bones": bones,
        })
    results = _run_cached(nc, in_maps)
    out = np.concatenate([results[c]["out"] for c in range(N_CORES)], axis=0)
    return out.astype(np.float32)


def _run_cached(nc, in_maps):
    """run_bass_kernel_spmd's axon path with the jitted executable cached
    across calls (avoids re-tracing the shard_map wrapper every call)."""
    import jax
    import numpy as _np
    from jax.sharding import Mesh, PartitionSpec
    from jax.experimental.shard_map import shard_map
    from concourse import bass2jax

    if "runner" not in _CACHED:
        bass2jax.install_neuronx_cc_hook()
        partition_name = (
            nc.partition_id_tensor.name if nc.partition_id_tensor else None
        )
        in_names, out_names, out_avals, zero_outs = [], [], [], []
        for alloc in nc.m.functions[0].allocations:
            if not isinstance(alloc, mybir.MemoryLocationSet):
                continue
            name = alloc.memorylocations[0].name
            if alloc.kind == "ExternalInput":
                if name != partition_name:
                    in_names.append(name)
            elif alloc.kind == "ExternalOutput":
                shape = tuple(alloc.tensor_shape)
                dtype = mybir.dt.np(alloc.dtype)
                out_names.append(name)
                out_avals.append(jax.core.ShapedArray(shape, dtype))
                zero_outs.append(_np.zeros(shape, dtype))
        n_params = len(in_names)
        n_outs = len(out_avals)
        all_names = list(in_names) + list(out_names)
        if partition_name is not None:
            all_names.append(partition_name)
        donate = tuple(range(n_params, n_params + n_outs))

        def _body(*args):
            operands = list(args)
            if partition_name is not None:
                operands.append(bass2jax.partition_id_tensor())
            outs = bass2jax._bass_exec_p.bind(
                *operands,
                out_avals=tuple(out_avals),
                in_names=tuple(all_names),
                out_names=tuple(out_names),
                lowering_input_output_aliases=(),
                sim_require_finite=True,
                sim_require_nnan=True,
                nc=nc,
            )
            return tuple(outs)

        devices = jax.devices()[:N_CORES]
        mesh = Mesh(_np.asarray(devices), ("core",))
        in_specs = (PartitionSpec("core"),) * (n_params + n_outs)
        out_specs = (PartitionSpec("core"),) * n_outs
        sharded = jax.jit(
            shard_map(_body, mesh=mesh, in_specs=in_specs,
                      out_specs=out_specs, check_rep=False),
            donate_argnums=donate, keep_unused=True,
        )
        _CACHED["runner"] = (sharded, in_names, out_names, out_avals, zero_outs)

    sharded, in_names, out_names, out_avals, zero_outs = _CACHED["runner"]
    import numpy as _np2
    concat_in = [
        _np2.concatenate([_np2.asarray(in_maps[c][n]) for c in range(N_CORES)], axis=0)
        for n in in_names
    ]
    concat_zeros = [
        _np2.zeros((N_CORES * z.shape[0], *z.shape[1:]), z.dtype) for z in zero_outs
    ]
    out_arrs = sharded(*concat_in, *concat_zeros)
    return [
        {
            name: _np2.asarray(out_arrs[i]).reshape(N_CORES, *out_avals[i].shape)[c]
            for i, name in enumerate(out_names)
        }
        for c in range(N_CORES)
    ]


if __name__ == "__main__":
    rng = np.random.default_rng(0)
    x = rng.standard_normal((B_FULL, CIN, H, W), dtype=np.float32)
    cw = rng.standard_normal((COUT, CIN, 3, 3), dtype=np.float32)
    out = kernel(x, cw, rng.standard_normal(COUT, dtype=np.float32),
                 rng.standard_normal(COUT, dtype=np.float32),
                 rng.standard_normal(COUT, dtype=np.float32),
                 rng.standard_normal((COUT, 1, 1), dtype=np.float32))
    print(out.shape, out.dtype)
